# revision 26
# baseline (speedup 1.0000x reference)
"""Masked multi-head attention on 8 Trainium2 NeuronCores.

Reference computation (fp32):
    qkv = x @ W_qkv + b_qkv ; split q,k,v ; 16 heads, dh=64
    attn = softmax(causal(q k^T / 8)) ; z = attn v ; out = z @ W_proj + b_proj

Sharding: tensor-parallel over heads. Core c owns heads {2c, 2c+1}
(columns 128c:128c+128 of each of the q/k/v blocks of W_qkv, rows
128c:128c+128 of W_proj). Each core computes its heads' attention and a
partial output projection; the host sums the 8 partials and adds b_proj.

Measured-on-HW design notes (TRN2, this container):
  - matmul streaming ~132-160ns per 512-col instruction when the weight
    (LoadStationary) reload is hidden: inside PSUM accumulation chains, or
    when the stationary operand repeats (walrus dedups the reload).
    Isolated start/stop matmuls pay a large LS penalty (fp32r ~+450ns,
    bf16 ~+150ns), so everything LS-exposed runs in bf16.
  - K=64 matmuls on PE half-tiles (0,0)/(64,0) execute pairwise
    concurrently; the two heads' score matmuls are laid out exactly so.
  - The Activation engine (exp, ~0.83ns/elem) is the second bottleneck;
    PSUM drains are rebalanced DVE/Act and the emission order
    software-pipelines batch b's Act-heavy attention against batch b+1's
    PE-heavy QKV and batch b's projection.

Kernel layout:
  - x arrives pre-transposed bf16: xt[b] = x[b].T; QKV matmuls contract d
    on partitions in 8-chunk accumulation chains; q^T/k^T/v^T land in
    [e, t] with the two heads stacked 64+64 on partitions.  q/k biases are
    added by DVE (tensor_scalar_add), v bias by Act.
  - Scores per 128-key block, transposed S^T[k,q], as concurrent
    half-tile pairs; exp(scale=1/8) on Act from PSUM into bf16 pt; causal
    masking multiplies 0/1 bf16 tiles on gpsimd for diagonal blocks only;
    fully-masked columns are trimmed from matmul/exp/mask/AV
    (N = 512-128g on the g-th diagonal sub-block).
  - AV: lhsT = [v | ones64] (128 wide), so zp[h] = [z^T ; den x64] — the
    denominator lands replicated on partitions 64:128 and
    DVE-reciprocal([64,512]) yields the normalizer directly, no broadcast.
    Diagonal column-trimming splits the final write per 128-col range so
    each column's last touch carries stop=True.
  - Output projection is flipped: lhsT = W_proj^T chunk, rhs = z2T,
    producing out^T [d, q] partials per q-block (emitted one qi behind
    attention so the z2T h1 SBUF->SBUF DMA has drained); host
    transposes/sums the [B, D, S] partials and adds b_proj.

The harness-visible entry point is kernel(**inputs) -> np.ndarray.
"""

import sys

sys.path.insert(0, "/opt/trn_rl_repo")

import ml_dtypes
import numpy as np

B = 2
S = 2048
D = 1024
NH = 16
DH = 64
NCORES = 8
TT = 512  # t-tile for qkv / q-tile for scores
NQI = S // TT  # 4
NKJ = S // 128  # 16

BF = ml_dtypes.bfloat16


def _legalize_multi_waits(nc, max_waits=1):
    """This container's walrus rejects >1 semaphore wait per instruction
    (CoreV3 setupSyncWait "Too many sync wait commands"). Hoist extras
    onto same-engine NOPs inserted right before the offending one."""
    import concourse.mybir as mybir

    n_fixed = 0
    for fn in nc.m.functions:
        for blk in fn.blocks:
            new_insts = []
            for inst in blk.instructions:
                si = inst.sync_info
                waits = list(si.on_wait) if si is not None else []
                if len(waits) > max_waits:
                    extra, keep = waits[:-max_waits], waits[-max_waits:]
                    k = 0
                    while extra:
                        chunk, extra = extra[:max_waits], extra[max_waits:]
                        new_insts.append(
                            mybir.InstNoOp(
                                name=f"{inst.name}-wsplit{k}",
                                engine=inst.engine,
                                ins=[],
                                outs=[],
                                sync_info=mybir.SyncInfo(on_wait=chunk, on_update=[]),
                            )
                        )
                        k += 1
                    inst.sync_info = mybir.SyncInfo(
                        on_wait=keep, on_update=list(si.on_update)
                    )
                    n_fixed += 1
                new_insts.append(inst)
            blk.instructions = new_insts
    return n_fixed


def build_module(reps: int = 1, cfg: dict | None = None):
    cfg = dict(cfg or {})
    trim = cfg.get("trim", True)      # trim fully-masked diagonal columns
    cpair = cfg.get("cpair", False)   # projection over qb-pairs, wp LS reuse
    msplit = cfg.get("msplit", False) # alternate diag masks DVE/Pool
    dmat = cfg.get("dmat", False)     # v_nat via DMA transpose
    pemask = cfg.get("pemask", True)  # fold causal mask into PSUM via PE
    # --- timing-analysis knobs (numerically WRONG; never in BEST_CFG) ---
    dbg_act = cfg.get("dbg_act", 1.0)   # fraction of exp columns actually run
    dbg_pe = cfg.get("dbg_pe", 1.0)     # fraction of qkv/proj matmul cols
    dbg_dve = cfg.get("dbg_dve", 1.0)   # fraction of drain-copy cols
    import concourse.bass as bass
    import concourse.mybir as mybir
    import concourse.tile as tile
    from concourse.bass import ts
    from concourse.masks import make_identity

    F32 = mybir.dt.float32
    F32R = mybir.dt.float32r
    BF16 = mybir.dt.bfloat16
    Identity = mybir.ActivationFunctionType.Identity
    Exp = mybir.ActivationFunctionType.Exp

    nc = bass.Bass(
        trn_type="TRN2", target_bir_lowering=False, debug=False, num_devices=NCORES
    )

    xt = nc.dram_tensor("xt", [B, D, S], BF16, kind="ExternalInput").ap()
    wq = nc.dram_tensor("wq", [8, 128, 128], BF16, kind="ExternalInput").ap()
    wk = nc.dram_tensor("wk", [8, 128, 128], BF16, kind="ExternalInput").ap()
    wv = nc.dram_tensor("wv", [8, 128, 128], BF16, kind="ExternalInput").ap()
    bq = nc.dram_tensor("bq", [128, 1], F32, kind="ExternalInput").ap()
    bk = nc.dram_tensor("bk", [128, 1], F32, kind="ExternalInput").ap()
    bv = nc.dram_tensor("bv", [128, 1], F32, kind="ExternalInput").ap()
    wp = nc.dram_tensor("wp", [128, 8, 128], BF16, kind="ExternalInput").ap()
    msk = nc.dram_tensor("msk", [4, 128, 2 * TT], BF16, kind="ExternalInput").ap()
    mb = nc.dram_tensor("mb", [128, 128], BF16, kind="ExternalInput").ap()
    out = nc.dram_tensor("out", [B, D, S], BF16, kind="ExternalOutput").ap()

    with tile.TileContext(nc) as tc:
        with (
            tc.tile_pool(name="const", bufs=1) as cpool,
            tc.tile_pool(name="xtp", bufs=2) as xtp,
            tc.tile_pool(name="wk2", bufs=2) as wk2,
            tc.tile_pool(name="ppt", bufs=4) as ppt,
            tc.tile_pool(name="prb", bufs=2) as prb,
            tc.tile_pool(name="pos", bufs=4) as pos,
            tc.tile_pool(name="pssc", bufs=2, space="PSUM") as pssc,
            tc.tile_pool(name="psz", bufs=2, space="PSUM") as psz,
            tc.tile_pool(name="psp", bufs=2, space="PSUM") as psp,
        ):
            # ---- constants (outside the timing loop) ----
            wq_t = cpool.tile([128, 8, 128], BF16, tag="wq")
            wk_t = cpool.tile([128, 8, 128], BF16, tag="wk")
            wv_t = cpool.tile([128, 8, 128], BF16, tag="wv")
            nc.sync.dma_start(wq_t[:], wq.rearrange("o p e -> p o e"))
            nc.sync.dma_start(wk_t[:], wk.rearrange("o p e -> p o e"))
            nc.sync.dma_start(wv_t[:], wv.rearrange("o p e -> p o e"))
            bq_t = cpool.tile([128, 1], F32, tag="bq")
            bk_t = cpool.tile([128, 1], F32, tag="bk")
            bv_t = cpool.tile([128, 1], F32, tag="bv")
            nc.sync.dma_start(bq_t[:], bq[:])
            nc.sync.dma_start(bk_t[:], bk[:])
            nc.sync.dma_start(bv_t[:], bv[:])
            wp_t = cpool.tile([128, 8, 128], BF16, tag="wp")
            nc.sync.dma_start(wp_t[:], wp[:])
            msk_t = cpool.tile([128, 4, 2 * TT], BF16, tag="msk")
            if not pemask:
                nc.sync.dma_start(msk_t[:], msk.rearrange("g p q -> p g q"))
            mb_t = cpool.tile([128, 128], BF16, tag="mb")
            nc.sync.dma_start(mb_t[:], mb[:])
            ident = cpool.tile([128, 128], F32, tag="ident")
            make_identity(nc, ident[:])
            identb = cpool.tile([128, 128], BF16, tag="identb")
            nc.vector.tensor_copy(identb[:], ident[:])
            vnat_c = [
                cpool.tile(
                    [128, NKJ, 2, 128], BF16, tag=f"vnatc{b}", name=f"vnatc{b}"
                )
                for b in range(B)
            ]
            for b in range(B):
                nc.vector.memset(vnat_c[b][:, :, :, 64:128], 1.0)

            St = [dict() for _ in range(B)]

            def a_load(b):
                st = St[b]
                st["xt_f"] = xtp.tile([128, 8, S], BF16, tag="xtf", name=f"xtf{b}")
                for dc in range(8):
                    nc.sync.dma_start(st["xt_f"][:, dc, :], xt[b, ts(dc, 128), :])

            def a_qkv_unit(b, u):
                st = St[b]
                xt_f = st["xt_f"]
                if u == 0:
                    st["qT"] = wk2.tile([128, S], BF16, tag="qT", name=f"qT{b}")
                    st["kT"] = wk2.tile([128, S], BF16, tag="kT", name=f"kT{b}")
                    st["vT"] = wk2.tile([128, S], BF16, tag="vT", name=f"vT{b}")
                qT, kT, vT = st["qT"], st["kT"], st["vT"]
                TTe = int(TT * dbg_pe)
                if u < NQI:
                    tt_ = u
                    ps_qk = pssc.tile([128, 1024], F32, tag="scores", name=f"psqk{b}")
                    for dc in range(8):
                        st_, sp_ = dc == 0, dc == 7
                        x_sl = xt_f[:, dc, TT * tt_ : TT * tt_ + TTe]
                        nc.tensor.matmul(
                            ps_qk[:, 0:TTe], wq_t[:, dc], x_sl, start=st_, stop=sp_
                        )
                        nc.tensor.matmul(
                            ps_qk[:, 512 : 512 + TTe],
                            wk_t[:, dc],
                            x_sl,
                            start=st_,
                            stop=sp_,
                        )
                    dw = int(TT * dbg_dve)
                    nc.vector.tensor_scalar_add(
                        qT[:, TT * tt_ : TT * tt_ + dw], ps_qk[:, 0:dw], bq_t[:]
                    )
                    nc.scalar.activation(
                        kT[:, TT * tt_ : TT * tt_ + dw],
                        ps_qk[:, 512 : 512 + dw],
                        Identity,
                        bias=bk_t[:],
                    )
                else:
                    tp = u - NQI
                    ps_v = pssc.tile([128, 1024], F32, tag="scores", name=f"psv{b}")
                    for dc in range(8):
                        st_, sp_ = dc == 0, dc == 7
                        for j in (0, 1):
                            nc.tensor.matmul(
                                ps_v[:, j * 512 : j * 512 + TTe],
                                wv_t[:, dc],
                                xt_f[:, dc, TT * (2 * tp + j) : TT * (2 * tp + j) + TTe],
                                start=st_,
                                stop=sp_,
                            )
                    for j in (0, 1):
                        nc.scalar.activation(
                            vT[:, ts(2 * tp + j, TT)],
                            ps_v[:, j * 512 : j * 512 + 512],
                            Identity,
                            bias=bv_t[:],
                        )

            def a_qkv(b):
                for u in range(NQI + 2):
                    a_qkv_unit(b, u)

            def a_vnat(b):
                st = St[b]
                vT = st["vT"]
                v_nat = vnat_c[b]
                st["v_nat"] = v_nat
                for i in range(NKJ):
                    if dmat:
                        nc.sync.dma_start_transpose(
                            v_nat[:, i, :, 0:64], vT[:, ts(i, 128)]
                        )
                    else:
                        ps_t = psp.tile([128, 128], BF16, tag="pp", name=f"tp{b}")
                        nc.tensor.transpose(
                            ps_t[:, 0:128], vT[:, ts(i, 128)], identb[:]
                        )
                        nc.vector.tensor_copy(
                            v_nat[:, i, :, 0:64],
                            ps_t[:, 0:128].rearrange("p (h e) -> p h e", h=2),
                        )

            def b_qi(b, qi):
                st = St[b]
                qT, kT, v_nat = st["qT"], st["kT"], st["v_nat"]
                if qi == 0:
                    st["z2T"] = wk2.tile([128, S], BF16, tag="z2T", name=f"z2T{b}")
                    st["stg"] = wk2.tile([64, S], BF16, tag="stage", name=f"stg{b}")
                z2T, stg = st["z2T"], st["stg"]
                n_kj = 4 * qi + 4
                zp0 = psz.tile([128, 512], F32, tag="zp", name=f"zp0_{b}")
                zp1 = psz.tile([128, 512], F32, tag="zp", name=f"zp1_{b}")
                zps = (zp0, zp1)
                for kj in range(n_kj):
                    g = kj - 4 * qi
                    c0 = 128 * g if (trim and g > 0) else 0
                    sc = pssc.tile([128, 1024], F32, tag="scores", name=f"sc{b}")
                    if pemask and g >= 0:
                        # Diagonal block: the causal boundary lives in the
                        # 128-col window [cm, cm+128).  Scores there get
                        # start=True/stop=False, then an identity matmul
                        # accumulates a constant 0/-1e9 triangle so exp
                        # yields exact zeros -- no post-exp mask needed.
                        cm = 128 * g
                        for h in (0, 1):
                            base = h * 512
                            nc.tensor.matmul(
                                sc[:, base + cm : base + cm + 128],
                                kT[ts(h, 64), ts(kj, 128)],
                                qT[ts(h, 64), TT * qi + cm : TT * qi + cm + 128],
                                start=True,
                                stop=False,
                                skip_group_check=True,
                            )
                            if cm + 128 < 512:
                                # start=False: the bank was cleared by the
                                # window matmul above, so this is a plain
                                # first write (has_written=0 -> overwrite).
                                nc.tensor.matmul(
                                    sc[:, base + cm + 128 : base + 512],
                                    kT[ts(h, 64), ts(kj, 128)],
                                    qT[ts(h, 64), TT * qi + cm + 128 : TT * qi + TT],
                                    start=False,
                                    stop=True,
                                    skip_group_check=True,
                                )
                        for h in (0, 1):
                            nc.tensor.matmul(
                                sc[:, h * 512 + cm : h * 512 + cm + 128],
                                identb[:],
                                mb_t[:],
                                start=False,
                                stop=True,
                                skip_group_check=True,
                            )
                    else:
                        for h in (0, 1):
                            nc.tensor.matmul(
                                sc[:, h * 512 + c0 : h * 512 + 512],
                                kT[ts(h, 64), ts(kj, 128)],
                                qT[ts(h, 64), TT * qi + c0 : TT * qi + TT],
                                start=True,
                                stop=True,
                            )
                    pt = ppt.tile([128, 1024], BF16, tag="pt", name=f"pt{b}")
                    meng = nc.vector if (msplit and g % 2 == 1) else nc.gpsimd
                    if c0 == 0:
                        if dbg_act < 1.0:
                            nc.scalar.activation(
                                pt[:, 0 : int(1024 * dbg_act)],
                                sc[:, 0 : int(1024 * dbg_act)],
                                Exp,
                                scale=0.125,
                            )
                        else:
                            nc.scalar.activation(pt[:], sc[:], Exp, scale=0.125)
                        if g >= 0 and not pemask:
                            meng.tensor_mul(pt[:], pt[:], msk_t[:, g, :])
                    else:
                        ptv = pt.rearrange("p (h q) -> p h q", h=2)
                        scv = sc.rearrange("p (h q) -> p h q", h=2)
                        ce = c0 + max(1, int((512 - c0) * dbg_act))
                        nc.scalar.activation(
                            ptv[:, :, c0:ce], scv[:, :, c0:ce], Exp, scale=0.125
                        )
                        if not pemask:
                            mkv = msk_t[:, g, :].rearrange("p (h q) -> p h q", h=2)
                            meng.tensor_mul(
                                ptv[:, :, c0:512], ptv[:, :, c0:512], mkv[:, :, c0:512]
                            )
                    for h in (0, 1):
                        st_ = kj == 0
                        if not trim or g < 0:
                            nc.tensor.matmul(
                                zps[h][:, :],
                                v_nat[:, kj, h, :],
                                pt[:, h * 512 : h * 512 + 512],
                                start=st_,
                                stop=(kj == n_kj - 1) if not trim else False,
                                skip_group_check=True,
                            )
                        else:
                            nc.tensor.matmul(
                                zps[h][:, c0 : c0 + 128],
                                v_nat[:, kj, h, :],
                                pt[:, h * 512 + c0 : h * 512 + c0 + 128],
                                start=st_,
                                stop=True,
                                skip_group_check=True,
                            )
                            if g < 3:
                                nc.tensor.matmul(
                                    zps[h][:, c0 + 128 : 512],
                                    v_nat[:, kj, h, :],
                                    pt[:, h * 512 + c0 + 128 : h * 512 + 512],
                                    start=st_,
                                    stop=False,
                                    skip_group_check=True,
                                )
                dw = int(TT * dbg_dve)
                for h in (0, 1):
                    rbs = prb.tile([64, 512], F32R, tag="rbs", name=f"rbs{b}")
                    with nc.allow_low_precision(reason="fp32r softmax recip"):
                        nc.vector.reciprocal(rbs[:, 0:dw], zps[h][64:128, 0:dw])
                    if h == 0:
                        nc.vector.tensor_mul(
                            z2T[0:64, TT * qi : TT * qi + dw],
                            zps[h][0:64, 0:dw],
                            rbs[:, 0:dw],
                        )
                    else:
                        nc.vector.tensor_mul(
                            stg[:, TT * qi : TT * qi + dw],
                            zps[h][0:64, 0:dw],
                            rbs[:, 0:dw],
                        )
                        nc.sync.dma_start(
                            z2T[64:128, ts(qi, TT)], stg[:, ts(qi, TT)]
                        )

            def c_qb(b, qb):
                st = St[b]
                z2T = st["z2T"]
                dw = int(512 * dbg_dve)
                TTe = int(TT * dbg_pe)
                for j2 in range(4):
                    os_ = pos.tile([128, 2, 512], BF16, tag="ostage", name=f"os{b}")
                    for jj in (0, 1):
                        j = 2 * j2 + jj
                        pp = psp.tile([128, 512], F32, tag="pp", name=f"pp{b}")
                        nc.tensor.matmul(
                            pp[:, 0:TTe],
                            wp_t[:, j],
                            z2T[:, TT * qb : TT * qb + TTe],
                            start=True,
                            stop=True,
                        )
                        if jj == 1:
                            nc.scalar.copy(os_[:, jj, 0:dw], pp[:, 0:dw])
                        else:
                            nc.vector.tensor_copy(os_[:, jj, 0:dw], pp[:, 0:dw])
                    nc.sync.dma_start(
                        out[b, ts(j2, 256), ts(qb, TT)].rearrange(
                            "(j p) q -> p j q", p=128
                        ),
                        os_[:],
                    )

            def c_pair(b, half):
                # projection for q-blocks (2*half, 2*half+1); wp chunk stays
                # stationary across the two q-blocks (LS dedup)
                st = St[b]
                z2T = st["z2T"]
                for j2 in range(4):
                    oss = [
                        pos.tile([128, 2, 512], BF16, tag="ostage", name=f"os{b}")
                        for _ in range(2)
                    ]
                    for jj in (0, 1):
                        j = 2 * j2 + jj
                        for qx in (0, 1):
                            qb = 2 * half + qx
                            pp = psp.tile([128, 512], F32, tag="pp", name=f"pp{b}")
                            nc.tensor.matmul(
                                pp[:],
                                wp_t[:, j],
                                z2T[:, ts(qb, TT)],
                                start=True,
                                stop=True,
                            )
                            if j2 % 2 == 1 and jj == 1 and qx == 1:
                                nc.scalar.copy(oss[qx][:, jj, :], pp[:])
                            else:
                                nc.vector.tensor_copy(oss[qx][:, jj, :], pp[:])
                    for qx in (0, 1):
                        qb = 2 * half + qx
                        nc.sync.dma_start(
                            out[b, ts(j2, 256), ts(qb, TT)].rearrange(
                                "(j p) q -> p j q", p=128
                            ),
                            oss[qx][:],
                        )

            def body():
                a_load(0)
                a_qkv(0)
                a_vnat(0)
                a_load(1)
                if cpair:
                    b_qi(0, 0)
                    b_qi(0, 1)
                    b_qi(0, 2)
                    c_pair(0, 0)
                    b_qi(0, 3)
                    c_pair(0, 1)
                    a_qkv(1)
                    a_vnat(1)
                    b_qi(1, 0)
                    b_qi(1, 1)
                    b_qi(1, 2)
                    c_pair(1, 0)
                    b_qi(1, 3)
                    c_pair(1, 1)
                else:
                    b_qi(0, 0)
                    b_qi(0, 1)
                    c_qb(0, 0)
                    b_qi(0, 2)
                    c_qb(0, 1)
                    b_qi(0, 3)
                    c_qb(0, 2)
                    a_qkv(1)
                    a_vnat(1)
                    c_qb(0, 3)
                    b_qi(1, 0)
                    b_qi(1, 1)
                    c_qb(1, 0)
                    b_qi(1, 2)
                    c_qb(1, 1)
                    b_qi(1, 3)
                    c_qb(1, 2)
                    c_qb(1, 3)

            unroll = int(cfg.get("unroll", 1))
            if reps % unroll != 0:
                unroll = 1
            if reps == 1:
                body()
            else:
                engs = (
                    mybir.EngineType.PE,
                    mybir.EngineType.Activation,
                    mybir.EngineType.DVE,
                    mybir.EngineType.SP,
                    mybir.EngineType.Pool,
                )
                with tc.For_i(0, reps // unroll, 1, hint_engines=engs):
                    for _ in range(unroll):
                        body()

    _legalize_multi_waits(nc)
    return nc


def _host_inputs(x, W_qkv, b_qkv, W_proj):
    """Full inputs -> list of per-core input dicts."""
    x = np.asarray(x, dtype=np.float32)
    W_qkv = np.asarray(W_qkv, dtype=np.float32)
    b_qkv = np.asarray(b_qkv, dtype=np.float32)
    W_proj = np.asarray(W_proj, dtype=np.float32)

    xt = np.ascontiguousarray(x.transpose(0, 2, 1)).astype(BF)  # [B, D, S]

    masks = np.empty((4, 128, 2 * TT), dtype=np.float32)
    qidx = np.arange(TT)[None, :]
    kidx = np.arange(128)[:, None]
    for g in range(4):
        m = (qidx >= kidx + 128 * g).astype(np.float32)
        masks[g] = np.concatenate([m, m], axis=1)
    masks = masks.astype(BF)

    # 0 where the (query >= key) diagonal window keeps the score, -1e9 where
    # it is causally masked; accumulated into score PSUM by an identity matmul.
    jidx = np.arange(128)[None, :]
    mbias = np.where(jidx >= kidx, 0.0, -1e9).astype(np.float32).astype(BF)

    in_maps = []
    for c in range(NCORES):
        cols = slice(128 * c, 128 * c + 128)
        in_maps.append(
            {
                "xt": xt,
                "wq": np.ascontiguousarray(W_qkv[:, 0:1024][:, cols])
                .reshape(8, 128, 128)
                .astype(BF),
                "wk": np.ascontiguousarray(W_qkv[:, 1024:2048][:, cols])
                .reshape(8, 128, 128)
                .astype(BF),
                "wv": np.ascontiguousarray(W_qkv[:, 2048:3072][:, cols])
                .reshape(8, 128, 128)
                .astype(BF),
                "bq": np.ascontiguousarray(b_qkv[0:1024][cols]).reshape(128, 1),
                "bk": np.ascontiguousarray(b_qkv[1024:2048][cols]).reshape(128, 1),
                "bv": np.ascontiguousarray(b_qkv[2048:3072][cols]).reshape(128, 1),
                "wp": np.ascontiguousarray(W_proj[cols, :])
                .reshape(128, 8, 128)
                .astype(BF),
                "msk": masks,
                "mb": mbias,
            }
        )
    return in_maps


_module_cache = {}


BEST_CFG = {"unroll": 2}


def _get_module(reps: int = 1):
    if reps not in _module_cache:
        _module_cache[reps] = build_module(reps, BEST_CFG)
    return _module_cache[reps]


def run_on_device(in_maps, reps: int = 1):
    from concourse.bass_utils import run_bass_kernel_spmd

    nc = _get_module(reps)
    return run_bass_kernel_spmd(nc, in_maps, core_ids=list(range(NCORES)), trace=False)


def kernel(x, W_qkv, b_qkv, W_proj, b_proj):
    in_maps = _host_inputs(x, W_qkv, b_qkv, W_proj)
    res = run_on_device(in_maps, reps=1)
    acc = np.zeros((B, D, S), dtype=np.float32)
    for c in range(NCORES):
        acc += np.asarray(res.results[c]["out"], dtype=np.float32)
    out = acc.transpose(0, 2, 1) + np.asarray(b_proj, dtype=np.float32)
    return np.ascontiguousarray(out)



# revision 27
# speedup vs baseline: 1.0358x; 1.0358x over previous
"""Masked multi-head attention on 8 Trainium2 NeuronCores.

Reference computation (fp32):
    qkv = x @ W_qkv + b_qkv ; split q,k,v ; 16 heads, dh=64
    attn = softmax(causal(q k^T / 8)) ; z = attn v ; out = z @ W_proj + b_proj

Sharding: tensor-parallel over heads. Core c owns heads {2c, 2c+1}
(columns 128c:128c+128 of each of the q/k/v blocks of W_qkv, rows
128c:128c+128 of W_proj). Each core computes its heads' attention and a
partial output projection; the host sums the 8 partials and adds b_proj.

Measured-on-HW design notes (TRN2, this container):
  - matmul streaming ~132-160ns per 512-col instruction when the weight
    (LoadStationary) reload is hidden: inside PSUM accumulation chains, or
    when the stationary operand repeats (walrus dedups the reload).
    Isolated start/stop matmuls pay a large LS penalty (fp32r ~+450ns,
    bf16 ~+150ns), so everything LS-exposed runs in bf16.
  - K=64 matmuls on PE half-tiles (0,0)/(64,0) execute pairwise
    concurrently; the two heads' score matmuls are laid out exactly so.
  - The Activation engine (exp, ~0.83ns/elem) is the second bottleneck;
    PSUM drains are rebalanced DVE/Act and the emission order
    software-pipelines batch b's Act-heavy attention against batch b+1's
    PE-heavy QKV and batch b's projection.

Kernel layout:
  - x arrives pre-transposed bf16: xt[b] = x[b].T; QKV matmuls contract d
    on partitions in 8-chunk accumulation chains; q^T/k^T/v^T land in
    [e, t] with the two heads stacked 64+64 on partitions.  q/k biases are
    added by DVE (tensor_scalar_add), v bias by Act.
  - Scores per 128-key block, transposed S^T[k,q], as concurrent
    half-tile pairs; exp(scale=1/8) on Act from PSUM into bf16 pt; causal
    masking multiplies 0/1 bf16 tiles on gpsimd for diagonal blocks only;
    fully-masked columns are trimmed from matmul/exp/mask/AV
    (N = 512-128g on the g-th diagonal sub-block).
  - AV: lhsT = [v | ones64] (128 wide), so zp[h] = [z^T ; den x64] — the
    denominator lands replicated on partitions 64:128 and
    DVE-reciprocal([64,512]) yields the normalizer directly, no broadcast.
    Diagonal column-trimming splits the final write per 128-col range so
    each column's last touch carries stop=True.
  - Output projection is flipped: lhsT = W_proj^T chunk, rhs = z2T,
    producing out^T [d, q] partials per q-block (emitted one qi behind
    attention so the z2T h1 SBUF->SBUF DMA has drained); host
    transposes/sums the [B, D, S] partials and adds b_proj.

The harness-visible entry point is kernel(**inputs) -> np.ndarray.
"""

import sys

sys.path.insert(0, "/opt/trn_rl_repo")

import ml_dtypes
import numpy as np

B = 2
S = 2048
D = 1024
NH = 16
DH = 64
NCORES = 8
TT = 512  # t-tile for qkv / q-tile for scores
NQI = S // TT  # 4
NKJ = S // 128  # 16

BF = ml_dtypes.bfloat16


def _legalize_multi_waits(nc, max_waits=1):
    """This container's walrus rejects >1 semaphore wait per instruction
    (CoreV3 setupSyncWait "Too many sync wait commands"). Hoist extras
    onto same-engine NOPs inserted right before the offending one."""
    import concourse.mybir as mybir

    n_fixed = 0
    for fn in nc.m.functions:
        for blk in fn.blocks:
            new_insts = []
            for inst in blk.instructions:
                si = inst.sync_info
                waits = list(si.on_wait) if si is not None else []
                if len(waits) > max_waits:
                    extra, keep = waits[:-max_waits], waits[-max_waits:]
                    k = 0
                    while extra:
                        chunk, extra = extra[:max_waits], extra[max_waits:]
                        new_insts.append(
                            mybir.InstNoOp(
                                name=f"{inst.name}-wsplit{k}",
                                engine=inst.engine,
                                ins=[],
                                outs=[],
                                sync_info=mybir.SyncInfo(on_wait=chunk, on_update=[]),
                            )
                        )
                        k += 1
                    inst.sync_info = mybir.SyncInfo(
                        on_wait=keep, on_update=list(si.on_update)
                    )
                    n_fixed += 1
                new_insts.append(inst)
            blk.instructions = new_insts
    return n_fixed


def build_module(reps: int = 1, cfg: dict | None = None):
    cfg = dict(cfg or {})
    trim = cfg.get("trim", True)      # trim fully-masked diagonal columns
    cpair = cfg.get("cpair", False)   # projection over qb-pairs, wp LS reuse
    msplit = cfg.get("msplit", False) # alternate diag masks DVE/Pool
    dmat = cfg.get("dmat", False)     # v_nat via DMA transpose
    pemask = cfg.get("pemask", True)  # fold causal mask into PSUM via PE
    # --- timing-analysis knobs (numerically WRONG; never in BEST_CFG) ---
    dbg_act = cfg.get("dbg_act", 1.0)   # fraction of exp columns actually run
    dbg_pe = cfg.get("dbg_pe", 1.0)     # fraction of qkv/proj matmul cols
    dbg_dve = cfg.get("dbg_dve", 1.0)   # fraction of drain-copy cols
    dbg_norm = cfg.get("dbg_norm", 1.0)  # fraction of recip/mul cols
    dbg_os = cfg.get("dbg_os", 1.0)      # fraction of os_ drain cols
    import concourse.bass as bass
    import concourse.mybir as mybir
    import concourse.tile as tile
    from concourse.bass import ts
    from concourse.masks import make_identity

    F32 = mybir.dt.float32
    F32R = mybir.dt.float32r
    BF16 = mybir.dt.bfloat16
    Identity = mybir.ActivationFunctionType.Identity
    Exp = mybir.ActivationFunctionType.Exp

    nc = bass.Bass(
        trn_type="TRN2", target_bir_lowering=False, debug=False, num_devices=NCORES
    )

    xt = nc.dram_tensor("xt", [B, D, S], BF16, kind="ExternalInput").ap()
    wq = nc.dram_tensor("wq", [8, 128, 128], BF16, kind="ExternalInput").ap()
    wk = nc.dram_tensor("wk", [8, 128, 128], BF16, kind="ExternalInput").ap()
    wv = nc.dram_tensor("wv", [8, 128, 128], BF16, kind="ExternalInput").ap()
    bq = nc.dram_tensor("bq", [128, 1], F32, kind="ExternalInput").ap()
    bk = nc.dram_tensor("bk", [128, 1], F32, kind="ExternalInput").ap()
    bv = nc.dram_tensor("bv", [128, 1], F32, kind="ExternalInput").ap()
    wp = nc.dram_tensor("wp", [128, 8, 128], BF16, kind="ExternalInput").ap()
    msk = nc.dram_tensor("msk", [4, 128, 2 * TT], BF16, kind="ExternalInput").ap()
    mb = nc.dram_tensor("mb", [128, 128], BF16, kind="ExternalInput").ap()
    out = nc.dram_tensor("out", [B, D, S], BF16, kind="ExternalOutput").ap()

    with tile.TileContext(nc) as tc:
        with (
            tc.tile_pool(name="const", bufs=1) as cpool,
            tc.tile_pool(name="xtp", bufs=2) as xtp,
            tc.tile_pool(name="wk2", bufs=2) as wk2,
            tc.tile_pool(name="ppt", bufs=4) as ppt,
            tc.tile_pool(name="prb", bufs=2) as prb,
            tc.tile_pool(name="pos", bufs=4) as pos,
            tc.tile_pool(name="pssc", bufs=2, space="PSUM") as pssc,
            tc.tile_pool(name="psz", bufs=2, space="PSUM") as psz,
            tc.tile_pool(name="psp", bufs=2, space="PSUM") as psp,
        ):
            # ---- constants (outside the timing loop) ----
            wq_t = cpool.tile([128, 8, 128], BF16, tag="wq")
            wk_t = cpool.tile([128, 8, 128], BF16, tag="wk")
            wv_t = cpool.tile([128, 8, 128], BF16, tag="wv")
            nc.sync.dma_start(wq_t[:], wq.rearrange("o p e -> p o e"))
            nc.sync.dma_start(wk_t[:], wk.rearrange("o p e -> p o e"))
            nc.sync.dma_start(wv_t[:], wv.rearrange("o p e -> p o e"))
            bq_t = cpool.tile([128, 1], F32, tag="bq")
            bk_t = cpool.tile([128, 1], F32, tag="bk")
            bv_t = cpool.tile([128, 1], F32, tag="bv")
            nc.sync.dma_start(bq_t[:], bq[:])
            nc.sync.dma_start(bk_t[:], bk[:])
            nc.sync.dma_start(bv_t[:], bv[:])
            wp_t = cpool.tile([128, 8, 128], BF16, tag="wp")
            nc.sync.dma_start(wp_t[:], wp[:])
            msk_t = cpool.tile([128, 4, 2 * TT], BF16, tag="msk")
            if not pemask:
                nc.sync.dma_start(msk_t[:], msk.rearrange("g p q -> p g q"))
            mb_t = cpool.tile([128, 128], BF16, tag="mb")
            nc.sync.dma_start(mb_t[:], mb[:])
            ident = cpool.tile([128, 128], F32, tag="ident")
            make_identity(nc, ident[:])
            identb = cpool.tile([128, 128], BF16, tag="identb")
            nc.vector.tensor_copy(identb[:], ident[:])
            vnat_c = [
                cpool.tile(
                    [128, NKJ, 2, 128], BF16, tag=f"vnatc{b}", name=f"vnatc{b}"
                )
                for b in range(B)
            ]
            for b in range(B):
                nc.vector.memset(vnat_c[b][:, :, :, 64:128], 1.0)

            St = [dict() for _ in range(B)]

            def a_load(b):
                st = St[b]
                st["xt_f"] = xtp.tile([128, 8, S], BF16, tag="xtf", name=f"xtf{b}")
                for dc in range(8):
                    nc.sync.dma_start(st["xt_f"][:, dc, :], xt[b, ts(dc, 128), :])

            def a_qkv_unit(b, u):
                st = St[b]
                xt_f = st["xt_f"]
                if u == 0:
                    st["qT"] = wk2.tile([128, S], BF16, tag="qT", name=f"qT{b}")
                    st["kT"] = wk2.tile([128, S], BF16, tag="kT", name=f"kT{b}")
                    st["vT"] = wk2.tile([128, S], BF16, tag="vT", name=f"vT{b}")
                qT, kT, vT = st["qT"], st["kT"], st["vT"]
                TTe = int(TT * dbg_pe)
                if u < NQI:
                    tt_ = u
                    ps_qk = pssc.tile([128, 1024], F32, tag="scores", name=f"psqk{b}")
                    for dc in range(8):
                        st_, sp_ = dc == 0, dc == 7
                        x_sl = xt_f[:, dc, TT * tt_ : TT * tt_ + TTe]
                        nc.tensor.matmul(
                            ps_qk[:, 0:TTe], wq_t[:, dc], x_sl, start=st_, stop=sp_
                        )
                        nc.tensor.matmul(
                            ps_qk[:, 512 : 512 + TTe],
                            wk_t[:, dc],
                            x_sl,
                            start=st_,
                            stop=sp_,
                        )
                    dw = int(TT * dbg_dve)
                    nc.vector.tensor_scalar_add(
                        qT[:, TT * tt_ : TT * tt_ + dw], ps_qk[:, 0:dw], bq_t[:]
                    )
                    nc.scalar.activation(
                        kT[:, TT * tt_ : TT * tt_ + dw],
                        ps_qk[:, 512 : 512 + dw],
                        Identity,
                        bias=bk_t[:],
                    )
                else:
                    tp = u - NQI
                    ps_v = pssc.tile([128, 1024], F32, tag="scores", name=f"psv{b}")
                    for dc in range(8):
                        st_, sp_ = dc == 0, dc == 7
                        for j in (0, 1):
                            nc.tensor.matmul(
                                ps_v[:, j * 512 : j * 512 + TTe],
                                wv_t[:, dc],
                                xt_f[:, dc, TT * (2 * tp + j) : TT * (2 * tp + j) + TTe],
                                start=st_,
                                stop=sp_,
                            )
                    for j in (0, 1):
                        nc.scalar.activation(
                            vT[:, ts(2 * tp + j, TT)],
                            ps_v[:, j * 512 : j * 512 + 512],
                            Identity,
                            bias=bv_t[:],
                        )

            def a_qkv(b):
                for u in range(NQI + 2):
                    a_qkv_unit(b, u)

            def a_vnat(b):
                st = St[b]
                vT = st["vT"]
                v_nat = vnat_c[b]
                st["v_nat"] = v_nat
                for i in range(NKJ):
                    if dmat:
                        nc.sync.dma_start_transpose(
                            v_nat[:, i, :, 0:64], vT[:, ts(i, 128)]
                        )
                    else:
                        ps_t = psp.tile([128, 128], BF16, tag="pp", name=f"tp{b}")
                        nc.tensor.transpose(
                            ps_t[:, 0:128], vT[:, ts(i, 128)], identb[:]
                        )
                        nc.vector.tensor_copy(
                            v_nat[:, i, :, 0:64],
                            ps_t[:, 0:128].rearrange("p (h e) -> p h e", h=2),
                        )

            def b_qi(b, qi):
                st = St[b]
                qT, kT, v_nat = st["qT"], st["kT"], st["v_nat"]
                if qi == 0:
                    st["z2T"] = wk2.tile([128, S], BF16, tag="z2T", name=f"z2T{b}")
                    st["stg"] = wk2.tile([64, S], BF16, tag="stage", name=f"stg{b}")
                z2T, stg = st["z2T"], st["stg"]
                n_kj = 4 * qi + 4
                zp0 = psz.tile([128, 512], F32, tag="zp", name=f"zp0_{b}")
                zp1 = psz.tile([128, 512], F32, tag="zp", name=f"zp1_{b}")
                zps = (zp0, zp1)
                for kj in range(n_kj):
                    g = kj - 4 * qi
                    c0 = 128 * g if (trim and g > 0) else 0
                    sc = pssc.tile([128, 1024], F32, tag="scores", name=f"sc{b}")
                    if pemask and g >= 0:
                        # Diagonal block: the causal boundary lives in the
                        # 128-col window [cm, cm+128).  Scores there get
                        # start=True/stop=False, then an identity matmul
                        # accumulates a constant 0/-1e9 triangle so exp
                        # yields exact zeros -- no post-exp mask needed.
                        cm = 128 * g
                        for h in (0, 1):
                            base = h * 512
                            nc.tensor.matmul(
                                sc[:, base + cm : base + cm + 128],
                                kT[ts(h, 64), ts(kj, 128)],
                                qT[ts(h, 64), TT * qi + cm : TT * qi + cm + 128],
                                start=True,
                                stop=False,
                                skip_group_check=True,
                            )
                            if cm + 128 < 512:
                                # start=False: the bank was cleared by the
                                # window matmul above, so this is a plain
                                # first write (has_written=0 -> overwrite).
                                nc.tensor.matmul(
                                    sc[:, base + cm + 128 : base + 512],
                                    kT[ts(h, 64), ts(kj, 128)],
                                    qT[ts(h, 64), TT * qi + cm + 128 : TT * qi + TT],
                                    start=False,
                                    stop=True,
                                    skip_group_check=True,
                                )
                        for h in (0, 1):
                            nc.tensor.matmul(
                                sc[:, h * 512 + cm : h * 512 + cm + 128],
                                identb[:],
                                mb_t[:],
                                start=False,
                                stop=True,
                                skip_group_check=True,
                            )
                    else:
                        for h in (0, 1):
                            nc.tensor.matmul(
                                sc[:, h * 512 + c0 : h * 512 + 512],
                                kT[ts(h, 64), ts(kj, 128)],
                                qT[ts(h, 64), TT * qi + c0 : TT * qi + TT],
                                start=True,
                                stop=True,
                            )
                    pt = ppt.tile([128, 1024], BF16, tag="pt", name=f"pt{b}")
                    meng = nc.vector if (msplit and g % 2 == 1) else nc.gpsimd
                    if c0 == 0:
                        if dbg_act < 1.0:
                            nc.scalar.activation(
                                pt[:, 0 : int(1024 * dbg_act)],
                                sc[:, 0 : int(1024 * dbg_act)],
                                Exp,
                                scale=0.125,
                            )
                        else:
                            nc.scalar.activation(pt[:], sc[:], Exp, scale=0.125)
                        if g >= 0 and not pemask:
                            meng.tensor_mul(pt[:], pt[:], msk_t[:, g, :])
                    else:
                        ptv = pt.rearrange("p (h q) -> p h q", h=2)
                        scv = sc.rearrange("p (h q) -> p h q", h=2)
                        ce = c0 + max(1, int((512 - c0) * dbg_act))
                        nc.scalar.activation(
                            ptv[:, :, c0:ce], scv[:, :, c0:ce], Exp, scale=0.125
                        )
                        if not pemask:
                            mkv = msk_t[:, g, :].rearrange("p (h q) -> p h q", h=2)
                            meng.tensor_mul(
                                ptv[:, :, c0:512], ptv[:, :, c0:512], mkv[:, :, c0:512]
                            )
                    for h in (0, 1):
                        st_ = kj == 0
                        if not trim or g < 0:
                            nc.tensor.matmul(
                                zps[h][:, :],
                                v_nat[:, kj, h, :],
                                pt[:, h * 512 : h * 512 + 512],
                                start=st_,
                                stop=(kj == n_kj - 1) if not trim else False,
                                skip_group_check=True,
                            )
                        else:
                            nc.tensor.matmul(
                                zps[h][:, c0 : c0 + 128],
                                v_nat[:, kj, h, :],
                                pt[:, h * 512 + c0 : h * 512 + c0 + 128],
                                start=st_,
                                stop=True,
                                skip_group_check=True,
                            )
                            if g < 3:
                                nc.tensor.matmul(
                                    zps[h][:, c0 + 128 : 512],
                                    v_nat[:, kj, h, :],
                                    pt[:, h * 512 + c0 + 128 : h * 512 + 512],
                                    start=st_,
                                    stop=False,
                                    skip_group_check=True,
                                )
                dw = int(TT * dbg_dve * dbg_norm)
                for h in (0, 1):
                    rbs = prb.tile([64, 512], F32R, tag="rbs", name=f"rbs{b}")
                    with nc.allow_low_precision(reason="fp32r softmax recip"):
                        nc.vector.reciprocal(rbs[:, 0:dw], zps[h][64:128, 0:dw])
                    if h == 0:
                        nc.vector.tensor_mul(
                            z2T[0:64, TT * qi : TT * qi + dw],
                            zps[h][0:64, 0:dw],
                            rbs[:, 0:dw],
                        )
                    else:
                        nc.vector.tensor_mul(
                            stg[:, TT * qi : TT * qi + dw],
                            zps[h][0:64, 0:dw],
                            rbs[:, 0:dw],
                        )
                        nc.sync.dma_start(
                            z2T[64:128, ts(qi, TT)], stg[:, ts(qi, TT)]
                        )

            def c_qb(b, qb):
                st = St[b]
                z2T = st["z2T"]
                dw = int(512 * dbg_dve * dbg_os)
                TTe = int(TT * dbg_pe)
                for j2 in range(4):
                    os_ = pos.tile([128, 2, 512], BF16, tag="ostage", name=f"os{b}")
                    for jj in (0, 1):
                        j = 2 * j2 + jj
                        pp = psp.tile([128, 512], F32, tag="pp", name=f"pp{b}")
                        nc.tensor.matmul(
                            pp[:, 0:TTe],
                            wp_t[:, j],
                            z2T[:, TT * qb : TT * qb + TTe],
                            start=True,
                            stop=True,
                        )
                        if jj == 1:
                            nc.scalar.copy(os_[:, jj, 0:dw], pp[:, 0:dw])
                        else:
                            nc.vector.tensor_copy(os_[:, jj, 0:dw], pp[:, 0:dw])
                    nc.sync.dma_start(
                        out[b, ts(j2, 256), ts(qb, TT)].rearrange(
                            "(j p) q -> p j q", p=128
                        ),
                        os_[:],
                    )

            def c_pair(b, half):
                # projection for q-blocks (2*half, 2*half+1); wp chunk stays
                # stationary across the two q-blocks (LS dedup)
                st = St[b]
                z2T = st["z2T"]
                for j2 in range(4):
                    oss = [
                        pos.tile([128, 2, 512], BF16, tag="ostage", name=f"os{b}")
                        for _ in range(2)
                    ]
                    for jj in (0, 1):
                        j = 2 * j2 + jj
                        for qx in (0, 1):
                            qb = 2 * half + qx
                            pp = psp.tile([128, 512], F32, tag="pp", name=f"pp{b}")
                            nc.tensor.matmul(
                                pp[:],
                                wp_t[:, j],
                                z2T[:, ts(qb, TT)],
                                start=True,
                                stop=True,
                            )
                            if j2 % 2 == 1 and jj == 1 and qx == 1:
                                nc.scalar.copy(oss[qx][:, jj, :], pp[:])
                            else:
                                nc.vector.tensor_copy(oss[qx][:, jj, :], pp[:])
                    for qx in (0, 1):
                        qb = 2 * half + qx
                        nc.sync.dma_start(
                            out[b, ts(j2, 256), ts(qb, TT)].rearrange(
                                "(j p) q -> p j q", p=128
                            ),
                            oss[qx][:],
                        )

            def body():
                a_load(0)
                a_qkv(0)
                a_vnat(0)
                a_load(1)
                if cpair:
                    b_qi(0, 0)
                    b_qi(0, 1)
                    b_qi(0, 2)
                    c_pair(0, 0)
                    b_qi(0, 3)
                    c_pair(0, 1)
                    a_qkv(1)
                    a_vnat(1)
                    b_qi(1, 0)
                    b_qi(1, 1)
                    b_qi(1, 2)
                    c_pair(1, 0)
                    b_qi(1, 3)
                    c_pair(1, 1)
                else:
                    b_qi(0, 0)
                    b_qi(0, 1)
                    c_qb(0, 0)
                    b_qi(0, 2)
                    c_qb(0, 1)
                    b_qi(0, 3)
                    c_qb(0, 2)
                    a_qkv(1)
                    a_vnat(1)
                    c_qb(0, 3)
                    b_qi(1, 0)
                    b_qi(1, 1)
                    c_qb(1, 0)
                    b_qi(1, 2)
                    c_qb(1, 1)
                    b_qi(1, 3)
                    c_qb(1, 2)
                    c_qb(1, 3)

            unroll = int(cfg.get("unroll", 1))
            if reps % unroll != 0:
                unroll = 1
            if reps == 1:
                body()
            else:
                engs = (
                    mybir.EngineType.PE,
                    mybir.EngineType.Activation,
                    mybir.EngineType.DVE,
                    mybir.EngineType.SP,
                    mybir.EngineType.Pool,
                )
                with tc.For_i(0, reps // unroll, 1, hint_engines=engs):
                    for _ in range(unroll):
                        body()

    _legalize_multi_waits(nc)
    return nc


def _host_inputs(x, W_qkv, b_qkv, W_proj):
    """Full inputs -> list of per-core input dicts."""
    x = np.asarray(x, dtype=np.float32)
    W_qkv = np.asarray(W_qkv, dtype=np.float32)
    b_qkv = np.asarray(b_qkv, dtype=np.float32)
    W_proj = np.asarray(W_proj, dtype=np.float32)

    xt = np.ascontiguousarray(x.transpose(0, 2, 1)).astype(BF)  # [B, D, S]

    masks = np.empty((4, 128, 2 * TT), dtype=np.float32)
    qidx = np.arange(TT)[None, :]
    kidx = np.arange(128)[:, None]
    for g in range(4):
        m = (qidx >= kidx + 128 * g).astype(np.float32)
        masks[g] = np.concatenate([m, m], axis=1)
    masks = masks.astype(BF)

    # 0 where the (query >= key) diagonal window keeps the score, -1e9 where
    # it is causally masked; accumulated into score PSUM by an identity matmul.
    jidx = np.arange(128)[None, :]
    mbias = np.where(jidx >= kidx, 0.0, -1e9).astype(np.float32).astype(BF)

    in_maps = []
    for c in range(NCORES):
        cols = slice(128 * c, 128 * c + 128)
        in_maps.append(
            {
                "xt": xt,
                "wq": np.ascontiguousarray(W_qkv[:, 0:1024][:, cols])
                .reshape(8, 128, 128)
                .astype(BF),
                "wk": np.ascontiguousarray(W_qkv[:, 1024:2048][:, cols])
                .reshape(8, 128, 128)
                .astype(BF),
                "wv": np.ascontiguousarray(W_qkv[:, 2048:3072][:, cols])
                .reshape(8, 128, 128)
                .astype(BF),
                "bq": np.ascontiguousarray(b_qkv[0:1024][cols]).reshape(128, 1),
                "bk": np.ascontiguousarray(b_qkv[1024:2048][cols]).reshape(128, 1),
                "bv": np.ascontiguousarray(b_qkv[2048:3072][cols]).reshape(128, 1),
                "wp": np.ascontiguousarray(W_proj[cols, :])
                .reshape(128, 8, 128)
                .astype(BF),
                "msk": masks,
                "mb": mbias,
            }
        )
    return in_maps


_module_cache = {}


BEST_CFG = {"unroll": 2}


def _get_module(reps: int = 1):
    if reps not in _module_cache:
        _module_cache[reps] = build_module(reps, BEST_CFG)
    return _module_cache[reps]


def run_on_device(in_maps, reps: int = 1):
    from concourse.bass_utils import run_bass_kernel_spmd

    nc = _get_module(reps)
    return run_bass_kernel_spmd(nc, in_maps, core_ids=list(range(NCORES)), trace=False)


def kernel(x, W_qkv, b_qkv, W_proj, b_proj):
    in_maps = _host_inputs(x, W_qkv, b_qkv, W_proj)
    res = run_on_device(in_maps, reps=1)
    acc = np.zeros((B, D, S), dtype=np.float32)
    for c in range(NCORES):
        acc += np.asarray(res.results[c]["out"], dtype=np.float32)
    out = acc.transpose(0, 2, 1) + np.asarray(b_proj, dtype=np.float32)
    return np.ascontiguousarray(out)

